# revision 1
# baseline (speedup 1.0000x reference)
"""AttentionBlock (GroupNorm + 8-head attention + proj + residual) on 8 TRN2 cores.

Sharding: data-parallel over batch B=8 -> one image per NeuronCore, weights
replicated, no collectives.

Fast path:
 - S = (a*q)k^T in bf16 (K=64 contraction, FWL keeps ldweights hidden)
 - exp via bit-trick: P_bits = int8(max(S + 32, 0)) viewed as fp8e4m3 == 2^S
   (the 8*log2(e) logit scale is folded into the Q weights on the host);
   conversions split across Scalar/Vector/GpSimd engines
 - H = V @ P via fp8 DoubleRow (s = 128 partitions x 2), rowsum via ones row
 - QKV / V^T / proj matmuls in bf16, GroupNorm via bn_stats + group-mask matmul
 - bf16 output + on-device residual (x + proj_b), host upcasts
"""
import sys
import types

import numpy as np
import ml_dtypes

import concourse.bass as bass
import concourse.tile as tile
from concourse import bacc, mybir
from concourse.bass_utils import run_bass_kernel_spmd

F32 = mybir.dt.float32
BF16 = mybir.dt.bfloat16
FP8 = mybir.dt.float8e4
I8 = mybir.dt.int8
I32 = mybir.dt.int32

B, C, N = 8, 512, 1024          # batch, channels, H*W
NH, HD = 8, 64                  # heads, head_dim
G, GS = 32, 16                  # groups, channels per group
EPS = 1e-5
NCORES = 8
CT = C // 128                   # 4 channel tiles
ST = N // 128                   # 8 s-tiles
NCH = 2                         # t-chunks of 512
VTC = 80                        # vt cols per head (64 v + ones + pad to 16B)
TRACE = False
DEBUG = False

LOG2E = float(np.log2(np.e))
A8 = 8.0 * LOG2E                # folded into q weights: S_psum = 8*log2e*logits
B8 = 32.0                       # exp-bias for fp8e4m3 bit pattern (2^-3 common factor)

# engine assignment for the 64 exp tiles (cycled)
EXP_CYCLE = ['s', 'v'] * 15 + ['s', 's']

_CACHE = {}


def _install_ntff_hook():
    if "antenv.axon_hooks" in sys.modules:
        return
    try:
        from trn_agent_boot.trn_boot import _ntff_profile_via_ctypes
        hook = _ntff_profile_via_ctypes("/opt/axon/libaxon_pjrt.so")
    except Exception:
        hook = None
    mod = types.ModuleType("antenv.axon_hooks")
    mod.get_axon_ntff_profile_hook = lambda: hook
    mod.set_axon_ntff_profile_hook = lambda h: None
    sys.modules["antenv.axon_hooks"] = mod


def build_nc(debug=False):
    nc = bacc.Bacc("TRN2", target_bir_lowering=False, debug=False,
                   num_devices=NCORES)
    x = nc.dram_tensor("x", (C, N), BF16, kind="ExternalInput").ap()
    qkvw = nc.dram_tensor("qkvw", (C, 3 * C), BF16, kind="ExternalInput").ap()
    pw = nc.dram_tensor("pw", (C, C), BF16, kind="ExternalInput").ap()
    gnw = nc.dram_tensor("gnw", (128, CT), F32, kind="ExternalInput").ap()
    gnb = nc.dram_tensor("gnb", (128, CT), F32, kind="ExternalInput").ap()
    pb = nc.dram_tensor("pb", (128, CT), F32, kind="ExternalInput").ap()
    mask = nc.dram_tensor("mask", (128, 128), F32, kind="ExternalInput").ap()
    ident = nc.dram_tensor("ident", (128, 128), BF16, kind="ExternalInput").ap()
    out = nc.dram_tensor("out", (C, N), BF16, kind="ExternalOutput").ap()
    rs_scr = nc.dram_tensor("rs_scr", (NH, N), F32).ap()  # internal scratch
    rq_scr = nc.dram_tensor("rq_scr", (4, 2 * N), F32).ap()  # pair rowsums
    rr_scr = nc.dram_tensor("rr_scr", (4, 2 * N), F32).ap()  # pair recips

    dbg = {}
    if debug:
        for name, shape in [("d_xn", (C, N)), ("d_vt", (128, ST * NH * VTC)),
                            ("d_h", (C, N))]:
            dbg[name] = nc.dram_tensor(name, shape, F32, kind="ExternalOutput").ap()

    x_t = x.rearrange("(t p) n -> p t n", p=128)
    qkvw_t = qkvw.rearrange("(t p) o -> p t o", p=128)
    pw_t = pw.rearrange("(t p) o -> p t o", p=128)
    out_t = out.rearrange("(t p) n -> p t n", p=128)

    with tile.TileContext(nc) as tc:
        with (
            tc.tile_pool(name="wpool", bufs=1) as wp,       # persistent
            tc.tile_pool(name="small", bufs=1) as sm,       # consts/stats
            tc.tile_pool(name="ppool", bufs=14) as pp,      # P fp8 tiles [128,2,1024]
            tc.tile_pool(name="hrawp", bufs=4) as hrawp,    # h_raw bf16 [64,1024]
            tc.tile_pool(name="rsp", bufs=3) as rsp,        # rowsum / recip [1,1024]
            tc.tile_pool(name="rsbp", bufs=3) as rsbp,      # broadcast [64,1024]
            tc.tile_pool(name="outp", bufs=2) as op_,       # output tiles
            tc.tile_pool(name="dbgp", bufs=1) as dbgp,      # debug dumps
            tc.tile_pool(name="ps_mm", bufs=2, space="PSUM") as ps_mm,
            tc.tile_pool(name="ps_s", bufs=2, space="PSUM") as ps_s,
            tc.tile_pool(name="ps_h", bufs=2, space="PSUM") as ps_h,
        ):
            # ---- persistent SBUF ----
            qkvw_sb = wp.tile([128, CT, 3 * C], BF16, tag="qkvw")
            pw_sb = wp.tile([128, CT, C], BF16, tag="pw")
            x_sb = wp.tile([128, CT, N], BF16, tag="xbf")
            xn_sb = wp.tile([128, CT, N], BF16, tag="xn")
            r_sb = wp.tile([128, CT, N], BF16, tag="res")
            q_sb = wp.tile([128, CT, N], BF16, tag="q")   # head-major [c, t]
            k_sb = wp.tile([128, CT, N], BF16, tag="k")
            vt_sb = wp.tile([128, ST, NH, VTC], FP8, tag="vt")
            h_sb = wp.tile([128, CT, N], BF16, tag="h")
            gnw_sb = wp.tile([128, CT], F32, tag="gnw")
            gnb_sb = wp.tile([128, CT], F32, tag="gnb")
            pb_sb = wp.tile([128, CT], F32, tag="pb")
            mask_sb = wp.tile([128, 128], F32, tag="mask")
            ident_sb = wp.tile([128, 128], BF16, tag="ident")

            # ---- input DMAs: x split fine on scalar/gpsimd queues; weights on sync ----
            for jh in range(2):
                nc.scalar.dma_start(out=x_sb[:, 0, jh * 512:(jh + 1) * 512],
                                    in_=x_t[:, 0, jh * 512:(jh + 1) * 512])
                nc.gpsimd.dma_start(out=x_sb[:, 1, jh * 512:(jh + 1) * 512],
                                    in_=x_t[:, 1, jh * 512:(jh + 1) * 512])
                nc.scalar.dma_start(out=x_sb[:, 2, jh * 512:(jh + 1) * 512],
                                    in_=x_t[:, 2, jh * 512:(jh + 1) * 512])
                nc.gpsimd.dma_start(out=x_sb[:, 3, jh * 512:(jh + 1) * 512],
                                    in_=x_t[:, 3, jh * 512:(jh + 1) * 512])
            nc.scalar.dma_start(out=qkvw_sb[:, :, 0:2 * C],
                                in_=qkvw_t[:, :, 0:2 * C])
            nc.sync.dma_start(out=mask_sb, in_=mask)
            nc.sync.dma_start(out=ident_sb, in_=ident)
            nc.sync.dma_start(out=gnw_sb, in_=gnw)
            nc.sync.dma_start(out=gnb_sb, in_=gnb)
            nc.sync.dma_start(out=pb_sb, in_=pb)
            nc.vector.memset(vt_sb[:, :, :, 64:65], 1.0)
            nc.vector.memset(vt_sb[:, :, :, 65:VTC], 0.0)

            ones_t = sm.tile([1, 64], F32, tag="ones")
            nc.vector.memset(ones_t, 1.0)
            eps_t = sm.tile([128, 1], F32, tag="eps")
            nc.vector.memset(eps_t, EPS)
            b8_t = sm.tile([128, 1], F32, tag="b8")
            nc.vector.memset(b8_t, B8)
            magic_t = sm.tile([128, 4], I32, tag="magic")
            nc.vector._memset_packed(magic_t, 0x5f3759df)

            # ---- GroupNorm stats (split vector / gpsimd) ----
            stats_in = sm.tile([128, 8], F32, tag="sin")
            for ct in range(CT):
                stats = sm.tile([128, 2, 6], F32, name=f"bst{ct}", tag="bst")
                for j in range(2):
                    nc.vector.bn_stats(out=stats[:, j, :],
                                       in_=x_sb[:, ct, j * 512:(j + 1) * 512])
                mv = sm.tile([128, 2], F32, name=f"mv{ct}", tag=f"mv{ct}")
                nc.vector.bn_aggr(out=mv, in_=stats)
                nc.vector.tensor_copy(stats_in[:, ct:ct + 1], mv[:, 0:1])
                msq = sm.tile([128, 1], F32, name=f"msq{ct}", tag=f"msq{ct}")
                nc.vector.tensor_mul(msq, mv[:, 0:1], mv[:, 0:1])
                nc.vector.tensor_add(stats_in[:, 4 + ct:5 + ct], mv[:, 1:2], msq)
            stats_ps = ps_mm.tile([128, 8], F32, tag="mm")
            nc.tensor.matmul(stats_ps, mask_sb, stats_in, start=True, stop=True)
            stats_gs = sm.tile([128, 8], F32, tag="sgs")
            nc.vector.tensor_copy(stats_gs, stats_ps)
            means_g = stats_gs[:, 0:4]
            e2_g = stats_gs[:, 4:8]
            msq_g = sm.tile([128, 4], F32, tag="msqg")
            nc.vector.tensor_mul(msq_g, means_g, means_g)
            var_g = sm.tile([128, 4], F32, tag="varg")
            nc.vector.tensor_tensor(out=var_g, in0=e2_g, in1=msq_g,
                                    op=mybir.AluOpType.subtract)
            # rstd = 1/sqrt(var+eps) via quake seed + 2 Newton iters (DVE only)
            veps = sm.tile([128, 4], F32, tag="veps")
            nc.vector.tensor_scalar(out=veps, in0=var_g, scalar1=EPS,
                                    scalar2=None, op0=mybir.AluOpType.add)
            yb = sm.tile([128, 4], I32, tag="yb")
            nc.vector.tensor_scalar(out=yb, in0=veps.bitcast(I32), scalar1=1,
                                    scalar2=None,
                                    op0=mybir.AluOpType.logical_shift_right)
            y0i = sm.tile([128, 4], I32, tag="y0i")
            nc.vector.tensor_tensor(out=y0i, in0=magic_t, in1=yb,
                                    op=mybir.AluOpType.subtract)
            rstd = y0i.bitcast(F32)
            for it in range(1):
                aa = sm.tile([128, 4], F32, name=f"nra{it}", tag=f"nra{it}")
                nc.vector.tensor_mul(aa, rstd, rstd)
                nc.vector.tensor_mul(aa, aa, veps)
                nc.vector.tensor_scalar(out=aa, in0=aa, scalar1=-0.5,
                                        scalar2=1.5, op0=mybir.AluOpType.mult,
                                        op1=mybir.AluOpType.add)
                nxt = sm.tile([128, 4], F32, name=f"nrn{it}", tag=f"nrn{it}")
                nc.vector.tensor_mul(nxt, rstd, aa)
                rstd = nxt
            sc_g = sm.tile([128, 4], F32, tag="scg")
            nc.vector.tensor_mul(sc_g, rstd, gnw_sb)
            tmp_b = sm.tile([128, 4], F32, tag="tmpb")
            nc.vector.tensor_mul(tmp_b, means_g, sc_g)
            bias_g = sm.tile([128, 4], F32, tag="biag")
            nc.vector.tensor_tensor(out=bias_g, in0=gnb_sb, in1=tmp_b,
                                    op=mybir.AluOpType.subtract)
            # xn (vector, 2x mode)
            for ct in range(CT):
                nc.vector.tensor_scalar(
                    out=xn_sb[:, ct, :], in0=x_sb[:, ct, :],
                    scalar1=sc_g[:, ct:ct + 1], scalar2=bias_g[:, ct:ct + 1],
                    op0=mybir.AluOpType.mult, op1=mybir.AluOpType.add)
            if debug:
                xn_f = dbgp.tile([128, N], F32, tag="dbgf")
                for ct in range(CT):
                    nc.vector.tensor_copy(xn_f, xn_sb[:, ct, :])
                    nc.sync.dma_start(out=dbg["d_xn"].rearrange(
                        "(t p) n -> p t n", p=128)[:, ct, :], in_=xn_f)

            # ---------------- emission helpers ----------------
            P = {}      # P[head][j] -> fp8 tile [128, 2, 1024] (j = st pair)
            rsf = {}    # rsf[head] -> f32 [1, 1024] rowsum
            rsr = {}    # (unused; recip is paired)
            rsb = {}    # broadcast [64, 1024]
            osb = {}

            def copy_on(e, out_, in_):
                if e == 's':
                    nc.scalar.activation(out=out_, in_=in_,
                                         func=mybir.ActivationFunctionType.Copy,
                                         bias=0.0, scale=1.0)
                elif e == 'g':
                    nc.gpsimd.tensor_copy(out_, in_)
                else:
                    nc.vector.tensor_copy(out_, in_)

            def exp_on(e, out_, in_):
                """out_bits = max(S + B8, 0) -> int8 == fp8e4m3 of 2^(S/A8*log2e)"""
                if e == 's':
                    nc.scalar.activation(out=out_, in_=in_,
                                         func=mybir.ActivationFunctionType.Relu,
                                         bias=b8_t, scale=1.0)
                else:
                    eng = nc.gpsimd if e == 'g' else nc.vector
                    eng.tensor_scalar(out=out_, in0=in_,
                                      scalar1=B8, scalar2=0.0,
                                      op0=mybir.AluOpType.add,
                                      op1=mybir.AluOpType.max)

            exp_i = [0]

            def next_exp_eng():
                e = EXP_CYCLE[exp_i[0] % len(EXP_CYCLE)]
                exp_i[0] += 1
                return e

            def qk_chain(pair, qk, ch, eng):
                """one QK chain -> psum -> bf16 cast into q_sb/k_sb."""
                dst = q_sb if qk == 0 else k_sb
                base = qk * C + pair * 128
                pt = ps_mm.tile([128, 512], F32, tag="mm")
                for kt in range(CT):
                    nc.tensor.matmul(
                        pt, qkvw_sb[:, kt, base:base + 128],
                        xn_sb[:, kt, ch * 512:(ch + 1) * 512],
                        start=(kt == 0), stop=(kt == CT - 1))
                copy_on(eng, dst[:, pair, ch * 512:(ch + 1) * 512], pt)

            def vt_mm(st, eng):
                pt = ps_mm.tile([128, 512], F32, tag="mm")
                for kt in range(CT):
                    nc.tensor.matmul(
                        pt, xn_sb[:, kt, st * 128:(st + 1) * 128],
                        qkvw_sb[:, kt, 2 * C:3 * C],
                        start=(kt == 0), stop=(kt == CT - 1))
                copy_on(eng, vt_sb[:, st, :, 0:64],
                        pt.rearrange("p (h c) -> p h c", h=NH))

            def s_exp(h, st):
                """bf16 S matmuls for (head, st) + exp bit-trick."""
                pair, lo = h // 2, (h % 2) * 64
                spt = ps_s.tile([128, N], F32, tag="s")
                for ch in range(NCH):
                    nc.tensor.matmul(
                        spt[:, ch * 512:(ch + 1) * 512],
                        k_sb[lo:lo + 64, pair, st * 128:(st + 1) * 128],
                        q_sb[lo:lo + 64, pair, ch * 512:(ch + 1) * 512],
                        start=True, stop=True)
                j, parity = st // 2, st % 2
                if j not in P.setdefault(h, {}):
                    P[h][j] = pp.tile([128, 2, N], FP8, name=f"P{h}_{j}", tag="P")
                exp_on(next_exp_eng(), P[h][j][:, parity, :].bitcast(I8), spt)

            hraw = {}   # hraw[head] -> bf16 [64, N]
            rsfp = {}   # rsfp[pair] -> f32 [2, N] rowsums for the head pair

            def h_mm(h, engs):
                """H DoubleRow chains -> hraw bf16 + rowsum rows into pair tile."""
                if h % 2 == 0:
                    rsfp[h // 2] = rsp.tile([1, 2 * N], F32, name=f"rsfp{h}", tag="rsfp")
                hraw[h] = hrawp.tile([64, N], BF16, name=f"hraw{h}", tag="hraw")
                for ch in range(NCH):
                    hpt = ps_h.tile([VTC, 512], F32, tag="hps")
                    for j in range(4):
                        nc.tensor.matmul(
                            hpt, vt_sb[:, 2 * j:2 * j + 2, h, :],
                            P[h][j][:, :, ch * 512:(ch + 1) * 512],
                            start=(j == 0), stop=(j == 3),
                            perf_mode=mybir.MatmulPerfMode.DoubleRow)
                    copy_on(engs[1 - ch],
                            rsfp[h // 2][:, (h % 2) * N + ch * 512:
                                         (h % 2) * N + (ch + 1) * 512],
                            hpt[64:65, :])
                    copy_on(engs[ch],
                            hraw[h][:, ch * 512:(ch + 1) * 512], hpt[0:64, :])

            def h_norm_pair(h0):
                """paired recip + per-head broadcast + norm."""
                pr = h0 // 2
                if h0 < 6:
                    # wide-recip: bounce rowsums via DRAM as [128,16] so the
                    # reciprocal uses all 128 lanes
                    nc.sync.dma_start(out=rq_scr[pr:pr + 1, :], in_=rsfp[pr])
                    rw = rsp.tile([128, 16], F32, name=f"rw{pr}", tag="rw")
                    nc.sync.dma_start(out=rw, in_=rq_scr[pr:pr + 1, :]
                                      .rearrange("o (p c) -> (o p) c", p=128))
                    rwr = rsp.tile([128, 16], F32, name=f"rwr{pr}", tag="rwr")
                    nc.vector.reciprocal_approx_fast(out=rwr, in_=rw)
                    nc.sync.dma_start(out=rr_scr[pr:pr + 1, :]
                                      .rearrange("o (p c) -> (o p) c", p=128),
                                      in_=rwr)
                    for i, h in enumerate((h0, h0 + 1)):
                        hrow = h_sb[(h % 2) * 64:(h % 2) * 64 + 64, h // 2, :]
                        rsb[h] = rsbp.tile([64, N], F32, name=f"rsb{h}", tag="rsb")
                        nc.sync.dma_start(
                            out=rsb[h],
                            in_=rr_scr[pr:pr + 1, i * N:(i + 1) * N]
                            .to_broadcast([64, N]))
                        nc.gpsimd.tensor_tensor(out=hrow, in0=hraw[h],
                                                in1=rsb[h],
                                                op=mybir.AluOpType.mult)
                else:
                    # tail heads 6,7 fast lane: per-head recip + mm-bcast + DVE
                    for i, h in enumerate((h0, h0 + 1)):
                        rsx = rsp.tile([1, N], F32, name=f"rsx{h}", tag=f"rsx{h}")
                        nc.vector.reciprocal_approx_fast(
                            out=rsx, in_=rsfp[pr][:, i * N:(i + 1) * N])
                        hrow = h_sb[(h % 2) * 64:(h % 2) * 64 + 64, h // 2, :]
                        for ch in range(NCH):
                            bpt = ps_h.tile([64, 512], F32, tag="hps")
                            nc.tensor.matmul(
                                bpt, ones_t, rsx[:, ch * 512:(ch + 1) * 512],
                                start=True, stop=True)
                            nc.vector.tensor_tensor(
                                out=hrow[:, ch * 512:(ch + 1) * 512],
                                in0=hraw[h][:, ch * 512:(ch + 1) * 512],
                                in1=bpt,
                                op=mybir.AluOpType.mult)


            def h_unit(h, engs):
                h_mm(h, engs)
                if h % 2 == 1:
                    h_norm_pair(h - 1)

            def proj_alloc(idx):
                if idx % 3 == 0:
                    return ps_mm.tile([128, 512], F32, name=f"pj{idx}", tag="mm")
                elif idx % 3 == 1:
                    ptw = ps_s.tile([128, N], F32, name=f"pjw{idx}", tag="s")
                    return ptw[:, 0:512]
                return ps_h.tile([128, 512], F32, name=f"pjh{idx}", tag="hps")

            def proj_head(pt, ot, ch):
                """seed with residual + kt0..kt2 accumulation."""
                nc.tensor.matmul(pt, ident_sb,
                                 r_sb[:, ot, ch * 512:(ch + 1) * 512],
                                 start=True, stop=False)
                for kt in range(CT - 1):
                    nc.tensor.matmul(
                        pt, pw_sb[:, kt, ot * 128:(ot + 1) * 128],
                        h_sb[:, kt, ch * 512:(ch + 1) * 512],
                        start=False, stop=False)

            def proj_tail(pt, ot, ch, idx):
                """kt3 + drain + out DMA."""
                nc.tensor.matmul(
                    pt, pw_sb[:, 3, ot * 128:(ot + 1) * 128],
                    h_sb[:, 3, ch * 512:(ch + 1) * 512],
                    start=False, stop=True)
                if ot not in osb:
                    osb[ot] = op_.tile([128, N], BF16, name=f"osb{ot}", tag="osb")
                copy_on('s' if idx % 2 == 0 else 'v',
                        osb[ot][:, ch * 512:(ch + 1) * 512], pt)
                if ch == NCH - 1:
                    nc.sync.dma_start(out=out_t[:, ot, :], in_=osb[ot])

            # ---------------- schedule ----------------
            # r0 gates the qkvw issue on the scalar queue so x transfers get
            # full DMA bandwidth first
            nc.scalar.activation(
                out=r_sb[:, 0, :], in_=x_sb[:, 0, :],
                func=mybir.ActivationFunctionType.Identity,
                bias=pb_sb[:, 0:1], scale=1.0)
            for ct in range(1, CT):
                nc.scalar.activation(
                    out=r_sb[:, ct, :], in_=x_sb[:, ct, :],
                    func=mybir.ActivationFunctionType.Identity,
                    bias=pb_sb[:, ct:ct + 1], scale=1.0)

            # Phase 1: QK pairs 0,1 (heads 0-3)
            ce = ['s', 'v']
            i = 0
            for pair in range(2):
                for qk in range(2):
                    for ch in range(NCH):
                        qk_chain(pair, qk, ch, ce[i % 2]); i += 1

            # late weight loads: v needed at VT (phase 2), pw at proj
            nc.sync.dma_start(out=qkvw_sb[:, :, 2 * C:3 * C],
                              in_=qkvw_t[:, :, 2 * C:3 * C])
            nc.sync.dma_start(out=pw_sb, in_=pw_t)

            # Phase 2: S(0), S(1) rounds + VT
            for st in range(ST):
                s_exp(0, st)
                s_exp(1, st)
                vt_mm(st, 's')

            # Phase 3: S(2), S(3) rounds + QK pairs 2,3 + H(0), H(1)
            qk1 = [(pair, qk, ch) for pair in (2, 3) for qk in range(2)
                   for ch in range(NCH)]
            for st in range(ST):
                s_exp(2, st)
                s_exp(3, st)
                pair, qk, ch = qk1[st]
                qk_chain(pair, qk, ch, 'v' if st % 2 == 0 else 's')
                if st == 2:
                    h_unit(0, ('s', 'v'))
                elif st == 6:
                    h_unit(1, ('s', 'v'))

            # Phase 4: S(4), S(5) rounds + QK rest + H(2), H(3)
            for st in range(ST):
                s_exp(4, st)
                s_exp(5, st)
                if st == 2:
                    h_unit(2, ('s', 'v'))
                elif st == 6:
                    h_unit(3, ('s', 'v'))

            # Phase 5: S(6), S(7) rounds + H(4), H(5) + projA
            for st in range(ST):
                s_exp(6, st)
                s_exp(7, st)
                if st == 0:
                    h_unit(4, ('s', 'v'))
                elif st == 2:
                    h_unit(5, ('s', 'v'))

            # Phase 6: H(6), H(7), proj
            h_unit(6, ('s', 'v'))
            h_unit(7, ('s', 'v'))
            chains = [(ot, ch) for ot in range(CT) for ch in range(NCH)]
            pts = {}
            for i in range(6):
                pts[i] = proj_alloc(i)
                proj_head(pts[i], *chains[i])
            for i in range(6):
                proj_tail(pts[i], *chains[i], i)
            for i in range(6, 8):
                pt = proj_alloc(i)
                proj_head(pt, *chains[i])
                proj_tail(pt, *chains[i], i)

            if debug:
                for ct in range(CT):
                    f = dbgp.tile([128, N], F32, tag="dbgh")
                    nc.vector.tensor_copy(f, h_sb[:, ct, :])
                    nc.sync.dma_start(out=dbg["d_h"].rearrange(
                        "(t p) n -> p t n", p=128)[:, ct, :], in_=f)

                for st in range(ST):
                    vf = dbgp.tile([128, NH * VTC], F32, name=f"vf{st}", tag="dbgf")
                    nc.vector.tensor_copy(
                        vf.rearrange("p (h c) -> p h c", h=NH), vt_sb[:, st, :, :])
                    nc.sync.dma_start(out=dbg["d_vt"].rearrange(
                        "p (s c) -> p s c", s=ST)[:, st, :], in_=vf)

    nc.finalize()
    return nc


def make_in_maps(x, gn_w, gn_b, qkv_w, proj_w, proj_b):
    x = np.asarray(x, dtype=np.float32).reshape(B, C, N)
    gn_w = np.asarray(gn_w, dtype=np.float32)
    gn_b = np.asarray(gn_b, dtype=np.float32)
    qkv_w = np.asarray(qkv_w, dtype=np.float32)
    proj_w = np.asarray(proj_w, dtype=np.float32)
    proj_b = np.asarray(proj_b, dtype=np.float32)

    scale = 1.0 / np.sqrt(np.sqrt(HD))
    rows = qkv_w.reshape(NH, 3, HD, C)
    qw = rows[:, 0].reshape(C, C) * (scale * A8)   # exp-scale folded
    kw = rows[:, 1].reshape(C, C) * scale
    vw = rows[:, 2].reshape(C, C)
    wall = np.concatenate([qw, kw, vw], axis=0)    # (3C, C)
    qkvw_t = np.ascontiguousarray(wall.T).astype(ml_dtypes.bfloat16)

    pw_t = np.ascontiguousarray(proj_w.T).astype(ml_dtypes.bfloat16)
    gnw_dev = np.ascontiguousarray(gn_w.reshape(CT, 128).T)
    gnb_dev = np.ascontiguousarray(gn_b.reshape(CT, 128).T)
    pb_dev = np.ascontiguousarray(proj_b.reshape(CT, 128).T)
    mask = np.zeros((128, 128), dtype=np.float32)
    for g in range(8):
        mask[g * GS:(g + 1) * GS, g * GS:(g + 1) * GS] = 1.0 / GS

    in_maps = []
    for b in range(B):
        xc = np.ascontiguousarray(x[b])
        in_maps.append({
            "x": xc.astype(ml_dtypes.bfloat16),
            "qkvw": qkvw_t, "pw": pw_t,
            "gnw": gnw_dev, "gnb": gnb_dev, "pb": pb_dev, "mask": mask,
            "ident": np.eye(128, dtype=ml_dtypes.bfloat16),
        })
    return in_maps


def kernel(x, gn_w, gn_b, qkv_w, proj_w, proj_b, num_heads):
    assert int(num_heads) == NH
    _install_ntff_hook()
    in_maps = make_in_maps(x, gn_w, gn_b, qkv_w, proj_w, proj_b)
    if "nc" not in _CACHE:
        _CACHE["nc"] = build_nc(debug=DEBUG)
    r = run_bass_kernel_spmd(_CACHE["nc"], in_maps,
                             core_ids=list(range(NCORES)), trace=TRACE)
    _CACHE["last_result"] = r
    out = np.stack([np.asarray(r.results[b]["out"], dtype=np.float32)
                    for b in range(B)])
    return out.reshape(B, C, 32, 32)

